# revision 5
# baseline (speedup 1.0000x reference)
"""Trainium2 Bass kernel for nn_BlockSampleFixed_47090021434001.

Reference semantics: for input (16, 64, 64, 64) f32, the output
(65536, 64, 4, 4) satisfies

    out[(b*64 + y)*64 + x, c, i, j] = in[b, c, y+i-3, x+j-2]

(zero outside bounds), with taps (i=3, j>=2) masked to zero — a 16-fold
shifted/zero-padded replication of the input transposed from
channel-major to pixel-major.

Strategy (pure data parallel, 2 batches per NeuronCore, no collectives):
  * Pure data movement, no arithmetic: every DMA'd byte is copied
    verbatim, so on-wire precision is the only lever. The per-core DMA
    ceiling is ~420 GB/s (16 DMA engines x ~26 GB/s, shared by all
    queues), so traffic is halved by moving fp16 on the device: the
    host preps fp16 slabs, the device stores an fp16 output, and the
    host converts back to f32 during the gather. Max relative error
    ~5e-4 (fp16 round-off), well inside the 2e-2 gate.
  * The host pre-builds, per core, a stacked slab tensor
        t2[(b,y) = 128 partitions, (d, c, xx) = 4*64*68] fp16
    where slab d is the input shifted down by d rows (zero-filled),
    x-padded by 3 left / 1 right (xx = x+3). Every tap (dy, dx) is
    then a pure free-dimension access on the device; partition-
    crossing work (the y-shifts and the c<->pixel transpose) never
    touches an engine.
  * On each core, for each of 8 x-tiles (8 pixels), the 14 live taps
    are interleaved into a pixel-major [128, 8*1024] fp16 tile with
    just 4 fused copies, one per filter row i: a hand-built source AP
    with overlapping windows (x and j both stride 1 in xx) maps
        src[p, x, c, j] = slab[3-i][p, c, x + j + 1]
    onto dst[p, x, c, 4i+j], so both sides' innermost dim is packed
    2-byte j — which qualifies for the DVE high-rate (2x/4x) modes
    and turns the strided scatter into 8-byte runs.  The two masked
    taps are memset once per rotating buffer.  The tile is stored
    with a single fully contiguous 2 MiB DMA, tiles alternating
    between the two HWDGE queues (SP + Activation).

The module also carries two workarounds for the walrus build in this
container, which rejects instructions carrying more than a few semaphore
waits ("Too many sync wait commands"): the TileContext final drain's
waits are split over sequencer NOPs, and a serialized-BIR rewrite moves
excess waits from any instruction onto injected same-engine NoOps.
"""

import json as _json

import numpy as np

import concourse.bass as bass
import concourse.mybir as mybir
import concourse.tile as tile
from concourse.vector_clock import ScopedClock, VectorClock

# ---------------------------------------------------------------------------
# walrus workaround #1: split the TileContext final-drain sem waits over
# several sequencer NOPs (<= 4 clock procs each).


def _split_drain_and_barrier(self, tick_clock, wait_clock):
    gclock = tick_clock.global_clock
    n = len(gclock)
    CHUNK = 4
    for start in range(0, n, CHUNK):
        vec = [0] * n
        nonzero = False
        for p in range(start, min(start + CHUNK, n)):
            t = gclock[p]
            vec[p] = t
            if t:
                nonzero = True
        if not nonzero:
            continue
        nop_inst = self.nc.sync.nop(nofuse=True, hint="drain_wait_split")
        wait_clock.add_sem_waits(nop_inst.ins, ScopedClock({None: VectorClock(vec)}))
    self.nc.sync.drain()
    self.nc.all_engine_barrier()
    popped = self.nc._tile_sem_poison_stack.pop()
    assert popped is self._sem_poison
    self.nc.clear_and_free_semaphores(list(self.sems.allocated().values()))
    self.nc.all_engine_barrier()


# ---------------------------------------------------------------------------
# walrus workaround #2: rewrite serialized BIR so no instruction carries
# more than one immediate sem wait; excess waits go to injected NoOps
# placed immediately before it (engine queues execute in list order).

_WSPLIT_KEEP = 1


def _split_bir_waits(bir_json):
    d = _json.loads(bir_json)
    n_new = 0
    for f in d.get("functions", []):
        for bb in f.get("blocks", []):
            insts = bb.get("instructions", [])
            out = []
            for inst in insts:
                si = inst.get("sync_info")
                waits = (si or {}).get("on_wait") or []
                movable = [w for w in waits if w.get("wait_reg") is None]
                fixed = [w for w in waits if w.get("wait_reg") is not None]
                nop_chunk = 1
                keep_limit = (
                    nop_chunk if inst.get("opcode") == "NoOp" else _WSPLIT_KEEP
                )
                if len(waits) > keep_limit:
                    keep_n = max(0, keep_limit - len(fixed))
                    keep, excess = movable[:keep_n], movable[keep_n:]
                    for i in range(0, len(excess), nop_chunk):
                        n_new += 1
                        out.append(
                            {
                                "debug": inst.get("debug"),
                                "engine": inst["engine"],
                                "ins": [],
                                "outs": [],
                                "name": f"I-wsplit-{n_new}",
                                "opcode": "NoOp",
                                "sync_info": {
                                    "on_update": [],
                                    "on_wait": excess[i:i + nop_chunk],
                                },
                                "text_hint": "wait_split",
                            }
                        )
                    si["on_wait"] = fixed + keep
                out.append(inst)
            bb["instructions"] = out
    enc = _json.dumps(d)
    return enc.encode() if isinstance(bir_json, bytes) else enc


_PATCHED = False


def _install_patches():
    global _PATCHED
    if _PATCHED:
        return
    tile.TileContext._drain_and_barrier = _split_drain_and_barrier

    import concourse.bass_utils as _bu
    import concourse.bass2jax as _b2j

    orig = _bu.compile_bir_kernel
    if not getattr(orig, "_wsplit_wrapped", False):

        def wrapper(bir_json, tmpdir, neff_name="file.neff"):
            return orig(_split_bir_waits(bir_json), tmpdir, neff_name=neff_name)

        wrapper._wsplit_wrapped = True
        _bu.compile_bir_kernel = wrapper
        _b2j.compile_bir_kernel = wrapper
    _PATCHED = True


# ---------------------------------------------------------------------------
# kernel proper

N_CORES = 8
B = 2            # batches per core (16 total / 8 cores)
C = 64
H = 64
W = 64
XX = 68          # padded width: xx = x + 3; pad cols {0,1,2,67} are zero
R = B * H        # 128 partition rows = (b, y)
SLABF = C * XX   # 4352 fp16 per slab per partition, (c, xx) xx-innermost
T2F = 4 * SLABF  # 4 stacked slabs
COLS = C * 16    # 1024 output columns per pixel
XT = 8           # pixels per output tile
F16 = mybir.dt.float16

from concourse.ap import AP as _AP


def _fused_src(t2v, d, x0, nj):
    """Overlapping-window AP src[p, x, c, j] = slab[d][p, c, x0+x+j+1]
    with x in [0, XT), j in [0, nj): x and j both stride 1 in xx."""
    base = t2v[:, d, :, x0 + 1:x0 + 1 + XT]  # (p, c, xx=XT) slice
    pairs = list(base.ap)                     # [(pstride,128),(XX,C),(1,XT)]
    pairs = [pairs[0], pairs[2], pairs[1], (1, nj)]  # (p, x, c, j)
    return _AP(base.tensor, base.offset, pairs)


def _build_nc():
    nc = bass.Bass()
    x = nc.dram_tensor("x", [R, T2F], F16, kind="ExternalInput")
    out = nc.dram_tensor("out", [B * H * W, COLS], F16, kind="ExternalOutput")
    nxt = W // XT

    with tile.TileContext(nc) as tc:
        with (
            tc.tile_pool(name="t2", bufs=1) as t2_pool,
            tc.tile_pool(name="outp", bufs=3) as out_pool,
        ):
            t2 = t2_pool.tile([R, T2F], F16, tag="t2", name="t2")
            t2v = t2[:].rearrange(
                "p (d c xx) -> p d c xx", d=4, c=C, xx=XX
            )
            xv = x.rearrange("p (d c xx) -> p d c xx", d=4, c=C, xx=XX)
            # one load per slab, round-robin over the two HWDGE queues;
            # d descending so tile copies (i ascending, d = 3 - i) chase
            # the loads and only the last half-group copy stays exposed
            qs = (nc.sync, nc.scalar)
            for d in (3, 2, 1, 0):
                qs[d % 2].dma_start(t2v[:, d], xv[:, d])

            for xt_i in range(nxt):
                x0 = xt_i * XT
                out_sb = out_pool.tile(
                    [R, XT * COLS], F16, tag="out_sb", name=f"out_sb_{xt_i}"
                )
                ov = out_sb[:].rearrange(
                    "p (x c s) -> p x c s", x=XT, c=C, s=16
                )
                # one fused copy per filter row i (d = 3 - i), j packed
                for i in range(4):
                    d = 3 - i
                    nj = 4 if i < 3 else 2
                    src = _fused_src(t2v, d, x0, nj)
                    dst = ov[:, :, :, 4 * i:4 * i + nj]
                    if i < 3:
                        nc.vector.tensor_copy(dst, src)
                    else:
                        nc.scalar.copy(dst, src)
                # masked taps: buffers rotate over 3; memset each once
                if xt_i < 3:
                    nc.gpsimd.memset(ov[:, :, :, 14:16], 0.0)
                dst = out.rearrange("(r x) n -> r x n", x=W)[:, x0:x0 + XT, :]
                qs[xt_i % 2].dma_start(dst, out_sb[:])

    return nc


def _host_prep(xb):
    """xb: (B, C, H, W) f32 core shard -> fp16 slab tensor [R, T2F]."""
    xbt = np.ascontiguousarray(
        xb.transpose(0, 2, 1, 3), dtype=np.float16
    )  # (b, y, c, x)
    t2 = np.zeros((B, H, 4, C, XX), dtype=np.float16)
    t2[:, :, 0, :, 3:3 + W] = xbt
    for d in (1, 2, 3):
        t2[:, d:, d, :, 3:3 + W] = xbt[:, :H - d]
    return t2.reshape(R, T2F)


_NC_CACHE = None


def kernel(inputs):
    """inputs: (16, 64, 64, 64) float32 -> (65536, 64, 4, 4) float32."""
    global _NC_CACHE
    _install_patches()
    from concourse.bass_utils import run_bass_kernel_spmd

    full = np.ascontiguousarray(np.asarray(inputs, dtype=np.float32))
    assert full.shape == (N_CORES * B, C, H, W), full.shape

    if _NC_CACHE is None:
        _NC_CACHE = _build_nc()
    nc = _NC_CACHE

    in_maps = [
        {"x": _host_prep(full[B * k:B * (k + 1)])} for k in range(N_CORES)
    ]
    res = run_bass_kernel_spmd(nc, in_maps, core_ids=list(range(N_CORES)))
    return np.concatenate(
        [np.asarray(res.results[k]["out"], dtype=np.float32)
         .reshape(B * H * W, C, 4, 4)
         for k in range(N_CORES)],
        axis=0,
    )
